# revision 1
# baseline (speedup 1.0000x reference)
"""Local+global causal self-attention (GQA + RMSNorm + RoPE) on 8 TRN2 cores.

Sharding: 8-way head-parallel. Core c owns q-heads {2c, 2c+1} which share
kv-head c//2 (GQA groups stay intact). Each core computes its 2 heads'
attention plus its slice of the output projection; the 8 partial outputs
(each summing 2 heads' contribution through Wo) are added on the host.
"""

import sys

sys.path.insert(0, "/opt/trn_rl_repo")

import json
import re

import numpy as np

import bass_rust
import concourse.bass as bass
import concourse.mybir as mybir
import concourse.tile as tile
from concourse.vector_clock import ScopedClock

P = 128
S = 2048
DIM = 1024
H = 16
KVH = 4
HD = 64
LW = 256
GT = 64
ROPE_BASE = 10000.0
N_CORES = 8
NQB = S // P  # 16 q blocks
F32 = mybir.dt.float32
F32R = mybir.dt.float32r
EPS = float(np.finfo(np.float32).eps)
SCALE = 1.0 / 8.0  # 1/sqrt(HD)


def _patched_drain_and_barrier(self, tick_clock, wait_clock):
    # This walrus build rejects >2 sem waits on a single Drain (TPB_CTRL).
    # Split the end-of-kernel waits across SP nops (<=1 wait each), then
    # drain bare. SP executes waits in program order, so the drain still
    # observes everything.
    gc = tick_clock.global_clock
    vals = [int(v) for v in re.findall(r"\d+", repr(gc))]
    for i, v in enumerate(vals):
        if v <= 0:
            continue
        sub = [0] * len(vals)
        sub[i] = v
        nop_inst = self.nc.sync.nop(nofuse=True)
        wait_clock.add_sem_waits(
            nop_inst.ins, ScopedClock({None: bass_rust.VectorClock(sub)})
        )
    self.nc.sync.drain()
    self.nc.all_engine_barrier()
    assert self.sems is not None
    popped = self.nc._tile_sem_poison_stack.pop()
    assert popped is self._sem_poison
    self.nc.clear_and_free_semaphores(list(self.sems.allocated().values()))
    self.nc.all_engine_barrier()


tile.TileContext._drain_and_barrier = _patched_drain_and_barrier

_MAXW = 1  # this walrus rejects >1 sync wait on one instruction


def _split_waits_json(raw: bytes) -> bytes:
    """Cap on_wait count per instruction; spill excess onto NoOps placed
    just before (same engine, executes its waits first in program order)."""
    m = json.loads(raw)
    ctr = 0
    for fn in m["functions"]:
        for bb in fn["blocks"]:
            out = []
            for ins in bb["instructions"]:
                si = ins.get("sync_info") or {}
                ow = si.get("on_wait") or []
                if len(ow) > _MAXW:
                    extra, keep = ow[:-_MAXW], ow[-_MAXW:]
                    for i in range(0, len(extra), _MAXW):
                        ctr += 1
                        out.append(
                            {
                                "debug": ins.get("debug", 0),
                                "engine": ins["engine"],
                                "ins": [],
                                "outs": [],
                                "name": f"I-wsp{ctr}",
                                "opcode": "NoOp",
                                "sync_info": {
                                    "on_update": [],
                                    "on_wait": extra[i : i + _MAXW],
                                },
                            }
                        )
                    si = dict(si)
                    si["on_wait"] = keep
                    ins = dict(ins)
                    ins["sync_info"] = si
                out.append(ins)
            bb["instructions"] = out
    return json.dumps(m).encode()


def _chunk_plan(i):
    """Key chunks attended by q block i: (key_block, n_keys, mask_idx).

    mask idx: 0 = tri (kk<=qq), 1 = band (kk>=qq+1), 2 = special i=2
    ((kk<64)|(kk>=qq+1)), None = fully allowed.
    """
    if i == 0:
        return [(0, 128, 0)]
    if i == 1:
        return [(0, 128, None), (1, 128, 0)]
    if i == 2:
        return [(0, 128, 2), (1, 128, None), (2, 128, 0)]
    return [(0, 64, None), (i - 2, 128, 1), (i - 1, 128, None), (i, 128, 0)]


def _r(ap):
    return ap.bitcast(F32R)


def build_nc():
    nc = bass.Bass()
    x_d = nc.dram_tensor("x", [S, DIM], F32, kind="ExternalInput")
    wq_d = nc.dram_tensor("wq", [DIM, P], F32, kind="ExternalInput")
    wkv_d = nc.dram_tensor("wkv", [DIM, P], F32, kind="ExternalInput")
    wo_d = nc.dram_tensor("wo", [P, DIM], F32, kind="ExternalInput")
    cc_d = nc.dram_tensor("cc", [HD, S], F32, kind="ExternalInput")
    ss_d = nc.dram_tensor("ss", [HD, S], F32, kind="ExternalInput")
    mk_d = nc.dram_tensor("mk", [P, 3, P], F32, kind="ExternalInput")
    id_d = nc.dram_tensor("ident", [P, P], F32, kind="ExternalInput")
    out_d = nc.dram_tensor("out", [S, DIM], F32, kind="ExternalOutput")

    DC = DIM // P  # 8 contraction chunks
    SB = 512  # moving-dim block for projections
    NSB = S // SB  # 4

    with (
        nc.allow_low_precision(reason="fp32r matmul input rounding"),
        tile.TileContext(nc) as tc,
    ):
        with tc.tile_pool(name="persist", bufs=1) as pp:
            # ---- persistent SBUF tensors ----
            xT = pp.tile([P, DC, S], F32, tag="xT")  # x transposed, 8MB
            qraw = pp.tile([HD, 2 * S], F32, tag="qraw")  # 2 heads wide
            qsw = pp.tile([HD, 2 * S], F32, tag="qsw")
            kvraw = pp.tile([P, S], F32, tag="kvraw")  # rows 0:64 kT, 64:128 vT
            ksw = pp.tile([HD, S], F32, tag="ksw")
            v_all = pp.tile([P, NQB, HD + 1], F32, tag="v_all")
            rfq = pp.tile([1, 2 * S], F32, tag="rfq")
            rfk = pp.tile([1, S], F32, tag="rfk")
            wq_sb = pp.tile([P, DC, P], F32, tag="wq")
            wkv_sb = pp.tile([P, DC, P], F32, tag="wkv")
            wo_sb = pp.tile([HD, 2, DIM], F32, tag="wo")  # heads on free dim
            cc_sb = pp.tile([HD, S], F32, tag="cc")
            ss_sb = pp.tile([HD, S], F32, tag="ss")
            mk_sb = pp.tile([P, 3, P], F32, tag="mk")
            id_sb = pp.tile([P, P], F32, tag="ident")

            nc.sync.dma_start(wq_sb[:].bitcast(F32R), wq_d.rearrange("(o p) m -> p o m", p=P).bitcast(F32R))
            nc.sync.dma_start(wkv_sb[:].bitcast(F32R), wkv_d.rearrange("(o p) m -> p o m", p=P).bitcast(F32R))
            nc.sync.dma_start(wo_sb[:].bitcast(F32R), wo_d.rearrange("(h p) n -> p h n", p=HD).bitcast(F32R))
            nc.scalar.dma_start(cc_sb[:], cc_d[:])
            nc.scalar.dma_start(ss_sb[:], ss_d[:])
            nc.sync.dma_start(mk_sb[:].bitcast(F32R), mk_d[:].bitcast(F32R))
            nc.scalar.dma_start(id_sb[:], id_d[:])
            nc.vector.memset(v_all[:, :, HD : HD + 1], 1.0)
            nc.vector.tensor_copy(
                v_all[:, :, HD : HD + 1].bitcast(F32R), v_all[:, :, HD : HD + 1]
            )
            eps_sb = pp.tile([1, 1], F32, tag="eps")
            nc.vector.memset(eps_sb[:], EPS)

            ones_row = mk_sb[0:1, 0, 0:HD]  # tri row 0 is all ones  [1, 64]
            ones_col = mk_sb[0:HD, 0, P - 1 : P]  # tri col 127 all ones [64, 1]

            # ---- phase 1+2: transpose x, project q/kv ----
            with (
                tc.tile_pool(name="xload", bufs=3) as xlp,
                tc.tile_pool(name="tp", bufs=2, space="PSUM") as tp,
                tc.tile_pool(name="projp", bufs=2, space="PSUM") as prp,
            ):
                cnt = 0
                for sc in range(S // P):  # 16 s tiles
                    xt = xlp.tile([P, DIM], F32, tag="x")
                    eng = nc.sync if sc % 2 == 0 else nc.gpsimd
                    eng.dma_start(xt[:], x_d[sc * P : (sc + 1) * P, :])
                    for dc in range(DC):
                        ps = tp.tile([P, P], F32, tag="tp")
                        nc.tensor.transpose(
                            ps[:], xt[:, dc * P : (dc + 1) * P], id_sb[:]
                        )
                        dst = xT[:, dc, sc * P : (sc + 1) * P].bitcast(F32R)
                        if cnt % 2 == 0:
                            nc.vector.tensor_copy(dst, ps[:])
                        else:
                            nc.scalar.copy(dst, ps[:])
                        cnt += 1

                for sb in range(NSB):
                    pq0 = prp.tile([HD, SB], F32, tag="q0")
                    pq1 = prp.tile([HD, SB], F32, tag="q1")
                    pkv = prp.tile([P, SB], F32, tag="kv")
                    for dc in range(DC):
                        rhs = _r(xT[:, dc, sb * SB : (sb + 1) * SB])
                        st, sp = dc == 0, dc == DC - 1
                        nc.tensor.matmul(
                            pq0[:], _r(wq_sb[:, dc, 0:HD]), rhs, start=st, stop=sp
                        )
                        nc.tensor.matmul(
                            pq1[:], _r(wq_sb[:, dc, HD:P]), rhs, start=st, stop=sp
                        )
                        nc.tensor.matmul(
                            pkv[:], _r(wkv_sb[:, dc, :]), rhs, start=st, stop=sp
                        )
                    nc.vector.tensor_copy(qraw[:, sb * SB : (sb + 1) * SB].bitcast(F32R), pq0[:])
                    nc.scalar.copy(qraw[:, S + sb * SB : S + (sb + 1) * SB].bitcast(F32R), pq1[:])
                    nc.vector.tensor_copy(kvraw[:, sb * SB : (sb + 1) * SB].bitcast(F32R), pkv[:])

            # ---- phase 3: rope swap copies, rms factors, rope+norm ----
            HF = HD // 2  # 32
            nc.gpsimd.dma_start(qsw[0:HF, :], qraw[HF:HD, :])
            nc.sync.dma_start(qsw[HF:HD, :], qraw[0:HF, :])
            nc.gpsimd.dma_start(ksw[0:HF, :], kvraw[HF:HD, :])
            nc.sync.dma_start(ksw[HF:HD, :], kvraw[0:HF, :])

            with (
                tc.tile_pool(name="sqp", bufs=3) as sqp,
                tc.tile_pool(name="rp", bufs=2, space="PSUM") as rp,
                tc.tile_pool(name="bcp", bufs=2, space="PSUM") as bcp,
                tc.tile_pool(name="vtp", bufs=2, space="PSUM") as vtp,
            ):
                # rsqrt(mean(t^2)+eps) factors for q (2 heads wide) and k
                for src, width, rf in ((qraw, 2 * S, rfq), (kvraw, S, rfk)):
                    for ch in range(width // SB):
                        sl = slice(ch * SB, (ch + 1) * SB)
                        sq = sqp.tile([HD, SB], F32, tag="sq")
                        nc.scalar.activation(
                            sq[:].bitcast(F32R),
                            src[0:HD, sl],
                            mybir.ActivationFunctionType.Square,
                        )
                        pr = rp.tile([1, SB], F32, tag="r")
                        nc.tensor.matmul(
                            pr[:], _r(ones_col), _r(sq[:]), start=True, stop=True
                        )
                        rt = sqp.tile([1, SB], F32, tag="rt")
                        nc.scalar.activation(
                            rt[:],
                            pr[:],
                            mybir.ActivationFunctionType.Sqrt,
                            bias=eps_sb[:],
                            scale=1.0 / HD,
                        )
                        nc.vector.reciprocal(rf[:, sl].bitcast(F32R), rt[:])

                # rope + norm, in place: t = (t*C + tsw*Ssgn) * rsqrt_bcast
                for src, sw, width, rf in (
                    (qraw, qsw, 2 * S, rfq),
                    (kvraw, ksw, S, rfk),
                ):
                    for ch in range(width // SB):
                        sl = slice(ch * SB, (ch + 1) * SB)
                        tsl = slice((ch * SB) % S, (ch * SB) % S + SB)
                        nc.vector.tensor_mul(src[0:HD, sl].bitcast(F32R), src[0:HD, sl], cc_sb[:, tsl])
                        nc.gpsimd.tensor_tensor(
                            sw[0:HD, sl], sw[0:HD, sl], ss_sb[:, tsl],
                            op=mybir.AluOpType.mult,
                        )
                        nc.vector.tensor_add(src[0:HD, sl].bitcast(F32R), src[0:HD, sl], sw[0:HD, sl])
                        pb = bcp.tile([HD, SB], F32, tag="bc")
                        nc.tensor.matmul(
                            pb[:], _r(ones_row), _r(rf[:, sl]), start=True, stop=True
                        )
                        nc.vector.tensor_mul(src[0:HD, sl].bitcast(F32R), src[0:HD, sl], pb[:])

                # ---- phase 4: vT -> v (normal layout) per key block ----
                for jb in range(NQB):
                    pv = vtp.tile([P, HD], F32, tag="vt")
                    nc.tensor.transpose(
                        pv[:],
                        kvraw[HD:P, jb * P : (jb + 1) * P],
                        id_sb[HD:P, HD:P],
                    )
                    nc.vector.tensor_copy(v_all[:, jb, 0:HD].bitcast(F32R), pv[:])

            # ---- phase 5+6: attention + output projection ----
            kT = kvraw  # rows 0:64 hold roped+normed kT
            q3 = qraw.rearrange("p (h s) -> p h s", h=2)  # [64, 2, 2048]
            with (
                tc.tile_pool(name="pattn", bufs=4) as pa,
                tc.tile_pool(name="ysb", bufs=2) as yp,
                tc.tile_pool(name="osb", bufs=2) as op,
                tc.tile_pool(name="psums", bufs=2, space="PSUM") as psp,
                tc.tile_pool(name="psumy", bufs=2, space="PSUM") as pyp,
                tc.tile_pool(name="psumb", bufs=1, space="PSUM") as pbp,
                tc.tile_pool(name="psumo", bufs=2, space="PSUM") as pop,
            ):
                for i in range(NQB):
                    plan = _chunk_plan(i)
                    q_i = _r(q3[:, :, i * P : (i + 1) * P])  # [64, 2, 128]
                    py = pyp.tile([HD + 1, 2 * P], F32, tag="y")
                    for ci, (jb, kn, mi) in enumerate(plan):
                        ps = psp.tile([P, 2 * P], F32, tag="s")
                        nc.tensor.matmul(
                            ps[0:kn, :],
                            _r(kT[0:HD, jb * P : jb * P + kn]),
                            q_i,
                            start=True,
                            stop=True,
                        )
                        pt = pa.tile([P, 2 * P], F32, tag="p")
                        nc.scalar.activation(
                            pt[0:kn, :].bitcast(F32R),
                            ps[0:kn, :],
                            mybir.ActivationFunctionType.Exp,
                            scale=SCALE,
                        )
                        if mi is not None:
                            pt3 = pt.rearrange("p (h q) -> p h q", h=2)
                            nc.gpsimd.tensor_tensor(
                                pt3[:].bitcast(F32R),
                                pt3[:],
                                mk_sb[:, mi : mi + 1, :].to_broadcast((P, 2, P)),
                                op=mybir.AluOpType.mult,
                            )
                        nc.tensor.matmul(
                            py[:],
                            _r(v_all[0:kn, jb, :]),
                            _r(pt[0:kn, :]),
                            start=(ci == 0),
                            stop=(ci == len(plan) - 1),
                        )
                    # normalize: y[d, c] * (1/den[c]) via PE broadcast
                    den = pa.tile([1, 2 * P], F32, tag="den")
                    nc.vector.reciprocal(den[:].bitcast(F32R), py[HD : HD + 1, :])
                    pb = pbp.tile([HD, 2 * P], F32, tag="ybc")
                    nc.tensor.matmul(
                        pb[:], _r(ones_row), _r(den[:]), start=True, stop=True
                    )
                    ysb = yp.tile([HD, 2 * P], F32, tag="ysb")
                    nc.scalar.copy(ysb[:].bitcast(F32R), py[0:HD, :])
                    nc.vector.tensor_mul(ysb[:].bitcast(F32R), ysb[:], pb[:])

                    # out projection: out[q, n] = sum_h yT_h.T @ Wo_h
                    osb = op.tile([P, DIM], F32, tag="osb")
                    for n2 in range(2):
                        po = pop.tile([P, SB], F32, tag="o")
                        for h in range(2):
                            nc.tensor.matmul(
                                po[:],
                                _r(ysb[:, h * P : (h + 1) * P]),
                                _r(wo_sb[:, h, n2 * SB : (n2 + 1) * SB]),
                                start=(h == 0),
                                stop=(h == 1),
                            )
                        if n2 == 0:
                            nc.vector.tensor_copy(osb[:, 0:SB], po[:])
                        else:
                            nc.scalar.copy(osb[:, SB : 2 * SB], po[:])
                    (nc.sync if i % 2 == 0 else nc.gpsimd).dma_start(out_d[i * P : (i + 1) * P, :], osb[:])

    return nc


def _host_constants():
    inv_freq = 1.0 / (ROPE_BASE ** (np.arange(0, HD, 2, dtype=np.float64) / HD))
    freqs = np.arange(S, dtype=np.float64)[:, None] * inv_freq[None, :]  # [S, 32]
    cos = np.cos(freqs).astype(np.float32).T  # [32, S]
    sin = np.sin(freqs).astype(np.float32).T
    cc = np.concatenate([cos, cos], axis=0)  # [64, S]
    ss = np.concatenate([sin, -sin], axis=0)

    kk = np.arange(P)[:, None]
    qq = np.arange(P)[None, :]
    tri = (kk <= qq).astype(np.float32)
    band = (kk >= qq + 1).astype(np.float32)
    sp2 = ((kk < GT) | (kk >= qq + 1)).astype(np.float32)
    mk = np.stack([tri, band, sp2], axis=1)  # [128, 3, 128]
    ident = np.eye(P, dtype=np.float32)
    return cc, ss, mk, ident


def kernel(x, Wq, Wk, Wv, Wo, profile=False):
    x = np.asarray(x, dtype=np.float32)
    Wq = np.asarray(Wq, dtype=np.float32)
    Wk = np.asarray(Wk, dtype=np.float32)
    Wv = np.asarray(Wv, dtype=np.float32)
    Wo = np.asarray(Wo, dtype=np.float32)
    bsz = x.shape[0]
    x2 = np.ascontiguousarray(x.reshape(S, DIM))

    cc, ss, mk, ident = _host_constants()
    in_maps = []
    for c in range(N_CORES):
        g = c // 2
        wq_c = np.ascontiguousarray(Wq[:, c * P : (c + 1) * P])
        wkv_c = np.ascontiguousarray(
            np.concatenate(
                [Wk[:, g * HD : (g + 1) * HD], Wv[:, g * HD : (g + 1) * HD]], axis=1
            )
        )
        wo_c = np.ascontiguousarray(Wo[c * P : (c + 1) * P, :])
        in_maps.append(
            {
                "x": x2,
                "wq": wq_c,
                "wkv": wkv_c,
                "wo": wo_c,
                "cc": cc,
                "ss": ss,
                "mk": mk,
                "ident": ident,
            }
        )

    from concourse import bass_utils
    from concourse.bass_utils import run_bass_kernel_spmd

    nc = build_nc()
    _orig_json = nc.to_json_bytes
    nc.to_json_bytes = lambda: _split_waits_json(_orig_json())
    exec_ns = None
    if profile:
        bass_utils.upload_artifacts = lambda tmpdir: tmpdir  # no bucket here
        try:
            res = run_bass_kernel_spmd(
                nc, in_maps, list(range(N_CORES)), trace=True
            )
            exec_ns = res.exec_time_ns
        except Exception as e:
            print("profile path failed, falling back:", repr(e))
            res = run_bass_kernel_spmd(nc, in_maps, list(range(N_CORES)))
    else:
        res = run_bass_kernel_spmd(nc, in_maps, list(range(N_CORES)))

    out = np.zeros((S, DIM), dtype=np.float32)
    for c in range(N_CORES):
        out += res.results[c]["out"]
    out = out.reshape(bsz, S, DIM)
    if profile:
        return out, exec_ns, res
    return out

